# revision 4
# baseline (speedup 1.0000x reference)
"""Trainium2 Bass kernel for nn_CustomFullyConnectedLayerGoogleTopK.

Reference computation:
    a = clip(K * softmax(alpha), 0, 1)                    # (4096,)
    W[rows, cols] += (V * a[:, None])  with rows=(j+i)%N, cols=j
    out = x @ W.T                                          # (256, 4096)

The scatter indices form a bijection (for each col j, row (j+i)%N hits every
row exactly once as i varies), so there is no actual accumulation:

    W[r, c] = V[(r - c) % N, c] * a[(r - c) % N]
    out[b, r] = sum_c x[b, c] * V[(r-c)%N, c] * a[(r-c)%N]

Sharding: output columns r are sharded 8 ways (512 per core) -> no collective;
each core reads only the diagonal band of V it needs (8 MB), all of x (4 MB),
and produces a disjoint out[:, r0:r0+512] slice.

Host-side prep is layout-only (static gather of V's wrapped diagonal band,
x transpose, alpha roll); softmax, clipping, scaling and the GEMM all run on
device. The per-core r0 offset is absorbed into the input layout (alpha is
rolled by r0) so all 8 cores run the same SPMD program.

The skewed scale field ab[c, j] = a[(j - c) % N] needed to scale band tiles is
Toeplitz, so a single SBUF master tile G[128, 4480] (loaded with one
negative-partition-stride DMA from a doubled copy of `a` in DRAM) provides the
scale for every c-block as a plain free-axis slice: block ca uses
G[:, 3968-ca : 4480-ca].

The GEMM runs in float32r (full-rate PE mode, ~1.5e-4 rms rel error measured
on HW vs fp64), accumulating fp32 in PSUM over all 32 c-blocks.
"""

import os
import sys

import numpy as np

for _p in ("/opt/trn_rl_repo", "/root/.axon_site/_ro/trn_rl_repo"):
    if os.path.isdir(_p) and _p not in sys.path:
        sys.path.append(_p)

import concourse.bacc as bacc
import concourse.bass as bass
import concourse.mybir as mybir
import concourse.tile as tile
from concourse.bass_utils import run_bass_kernel_spmd

F32 = mybir.dt.float32
F32R = mybir.dt.float32r

N = 4096          # IN_F == OUT_F == N_PERM == DIAG
B = 256           # batch
NCORES = 8
RW = N // NCORES  # 512 output columns per core
K_TOPK = 3687     # ceil(0.9 * 4096 * 4096 / 4096)
CB = 128          # contraction block
NCB = N // CB     # 32
GW = N + RW - CB  # 4480: width of the Toeplitz scale master tile


def _build_program():
    nc = bacc.Bacc("TRN2", target_bir_lowering=False, debug=False)

    band = nc.dram_tensor("band", [N, RW], F32, kind="ExternalInput").ap()
    xT = nc.dram_tensor("xT", [N, B], F32, kind="ExternalInput").ap()
    alpha_r = nc.dram_tensor("alpha_r", [N], F32, kind="ExternalInput").ap()
    out = nc.dram_tensor("out", [B, RW], F32, kind="ExternalOutput").ap()

    with tile.TileContext(nc) as tc:
        with (
            tc.tile_pool(name="small", bufs=1) as sp,
            tc.tile_pool(name="gpool", bufs=1) as gp,
            tc.tile_pool(name="dram", bufs=1, space="DRAM") as dp,
            tc.tile_pool(name="vb", bufs=6) as vbp,
            tc.tile_pool(name="wt", bufs=6) as wtp,
            tc.tile_pool(name="xt", bufs=6) as xtp,
            tc.tile_pool(name="opool", bufs=2) as op,
            tc.tile_pool(name="psum", bufs=1, space="PSUM") as pp,
            tc.tile_pool(name="psum_s", bufs=1, space="PSUM") as pps,
        ):
            # ---- Phase 0: a = min(K * softmax(alpha), 1); build G ----
            alpha_sb = sp.tile([128, N // 128], F32)
            nc.sync.dma_start(alpha_sb[:], alpha_r.rearrange("(p f) -> p f", p=128))

            exp_sb = sp.tile([128, N // 128], F32)
            rowsum = sp.tile([128, 1], F32)
            # alpha is uniform in [0,1): no max-subtraction needed for stability
            nc.scalar.activation(
                exp_sb[:], alpha_sb[:], mybir.ActivationFunctionType.Exp,
                accum_out=rowsum[:],
            )

            ones = sp.tile([128, 128], F32)
            nc.vector.memset(ones[:], 1.0)
            tot_ps = pps.tile([128, 1], F32)
            # total = ones.T @ rowsum -> per-partition copy of the full sum
            nc.tensor.matmul(tot_ps[:], ones[:], rowsum[:], start=True, stop=True)
            inv_sb = sp.tile([128, 1], F32)
            nc.vector.reciprocal(inv_sb[:], tot_ps[:])

            a_sb = sp.tile([128, N // 128], F32)
            nc.vector.tensor_scalar(
                a_sb[:], exp_sb[:], inv_sb[:], float(K_TOPK),
                mybir.AluOpType.mult, mybir.AluOpType.mult,
            )
            nc.vector.tensor_scalar_min(a_sb[:], a_sb[:], 1.0)

            # doubled copy of a in DRAM so the Toeplitz windows never wrap
            a2 = dp.tile([2 * N], F32)
            nc.sync.dma_start(a2[0:N].rearrange("(p f) -> p f", p=128), a_sb[:])
            nc.sync.dma_start(a2[N:2 * N].rearrange("(p f) -> p f", p=128), a_sb[:])

            # The c-axis is flipped within each 128-block on the host (band and
            # xT consistently), which turns the per-partition scale shift into
            # an ASCENDING Toeplitz: scale[ca+c_l, j] = a2[(3969 - ca) + c_l + j].
            # Master tile: G[c_l, t] = a2[1 + c_l + t]; block ca's scale is the
            # slice G[:, 3968-ca : 4480-ca]. (DMA partition steps must be >= 0,
            # hence the flip; overlapping +1-step windows are legal.)
            g_tile = gp.tile([128, GW], F32)
            g_src = bass.AP(a2.tensor, a2.offset + 1, [[1, 128], [1, GW]])
            nc.gpsimd.dma_start(g_tile[:], g_src)

            # ---- Phase 1: 32 c-blocks of band*scale -> 2 accumulating matmuls ----
            psum0 = pp.tile([128, RW], F32)
            psum1 = pp.tile([128, RW], F32)
            for k in range(NCB):
                ca = k * CB
                vb = vbp.tile([128, RW], F32)
                nc.sync.dma_start(vb[:], band[ca:ca + CB, :])
                xt = xtp.tile([128, B], F32R)
                nc.gpsimd.dma_start(xt[:], xT[ca:ca + CB, :])  # SWDGE cast f32->f32r

                wt = wtp.tile([128, RW], F32R)
                nc.vector.tensor_tensor(
                    wt[:], vb[:], g_tile[:, N - CB - ca:N - CB - ca + RW],
                    mybir.AluOpType.mult,
                )

                nc.tensor.matmul(psum0[:], xt[:, 0:128], wt[:],
                                 start=(k == 0), stop=(k == NCB - 1))
                nc.tensor.matmul(psum1[:], xt[:, 128:256], wt[:],
                                 start=(k == 0), stop=(k == NCB - 1))

            # ---- Phase 2: PSUM -> SBUF -> DRAM ----
            o0 = op.tile([128, RW], F32)
            nc.vector.tensor_copy(o0[:], psum0[:])
            nc.sync.dma_start(out[0:128, :], o0[:])
            o1 = op.tile([128, RW], F32)
            nc.vector.tensor_copy(o1[:], psum1[:])
            nc.sync.dma_start(out[128:256, :], o1[:])

    nc.compile()
    return nc


_NC_CACHE = []


def _get_program():
    if not _NC_CACHE:
        _NC_CACHE.append(_build_program())
    return _NC_CACHE[0]


def prepare_in_maps(x: np.ndarray, V: np.ndarray, alpha: np.ndarray):
    """Layout-only sharding of the full inputs into 8 per-core input maps."""
    x = np.ascontiguousarray(np.asarray(x, dtype=np.float32))
    V = np.ascontiguousarray(np.asarray(V, dtype=np.float32))
    alpha = np.ascontiguousarray(np.asarray(alpha, dtype=np.float32))

    def block_flip(m2d):
        # reverse row order within each 128-row block (see G-tile comment)
        return np.ascontiguousarray(
            m2d.reshape(NCB, CB, m2d.shape[1])[:, ::-1, :].reshape(m2d.shape)
        )

    xT = block_flip(x.T)  # (N, B)

    # VtD[c, t] = V[t % N, c] for t in [0, 2N): doubled transpose for wrap-free
    # band extraction. band_m[c, j] = V[(r0 + j - c) % N, c]
    #              = VtD[c, N + r0 + j - c]
    Vt = np.ascontiguousarray(V.T)
    VtD = np.concatenate([Vt, Vt], axis=1)  # (N, 2N)
    flat = VtD.reshape(-1)
    isz = flat.itemsize

    in_maps = []
    for m in range(NCORES):
        r0 = m * RW
        start = N + r0  # element offset of band_m[0, 0] in flat
        band_m = np.lib.stride_tricks.as_strided(
            flat[start:], shape=(N, RW), strides=((2 * N - 1) * isz, isz),
        )
        in_maps.append({
            "band": block_flip(band_m),
            "xT": xT,
            "alpha_r": np.ascontiguousarray(np.roll(alpha, -r0)),
        })
    return in_maps


def gather_output(results) -> np.ndarray:
    return np.concatenate([results[m]["out"] for m in range(NCORES)], axis=1)


def kernel(x: np.ndarray, V: np.ndarray, alpha: np.ndarray) -> np.ndarray:
    in_maps = prepare_in_maps(x, V, alpha)
    nc = _get_program()
    res = run_bass_kernel_spmd(nc, in_maps, core_ids=list(range(NCORES)))
    return gather_output(res.results)


# revision 6
# speedup vs baseline: 1.0937x; 1.0937x over previous
"""Trainium2 Bass kernel for nn_CustomFullyConnectedLayerGoogleTopK.

Reference computation:
    a = clip(K * softmax(alpha), 0, 1)                    # (4096,)
    W[rows, cols] += (V * a[:, None])  with rows=(j+i)%N, cols=j
    out = x @ W.T                                          # (256, 4096)

The scatter indices form a bijection (for each col j, row (j+i)%N hits every
row exactly once as i varies), so there is no actual accumulation:

    W[r, c] = V[(r - c) % N, c] * a[(r - c) % N]
    out[b, r] = sum_c x[b, c] * V[(r-c)%N, c] * a[(r-c)%N]

Sharding: output columns r are sharded 8 ways (512 per core) -> no collective;
each core reads only the diagonal band of V it needs (8 MB), all of x (4 MB),
and produces a disjoint out[:, r0:r0+512] slice.

Host-side prep is layout-only (static gather of V's wrapped diagonal band,
x transpose, row reversal, alpha roll); softmax, clipping, scaling and the
GEMM all run on device. The per-core r0 offset is absorbed into the input
layout (alpha is rolled by r0) so all 8 cores run the same SPMD program.

Device-side layout trick: with the contraction rows presented in REVERSED
order (c = N-1-p for SBUF partition-row p), the skewed scale field the band
tiles need becomes the ascending Toeplitz  scale[p, j] = a2[1 + p + j]  where
a2 is `a` doubled in DRAM. One master SBUF tile G[c_l, t] = a2[1 + c_l + t]
(loaded with a single overlapping-window DMA, partition step +1) then serves
every 128-row block q as the plain slice G[:, q*128 : q*128+512], so blocks
can be processed in batches of 4 with one strided DVE multiply each.

The GEMM runs in float32r (full-rate PE mode, ~1.5e-4 rms rel error measured
on HW vs fp64), accumulating fp32 in PSUM over all 32 c-blocks. The xT input
is declared float32r directly (same 4-byte layout; PE rounds internally) so
its tiles load via plain HWDGE DMA with no cast pass.
"""

import os
import sys

import numpy as np

for _p in ("/opt/trn_rl_repo", "/root/.axon_site/_ro/trn_rl_repo"):
    if os.path.isdir(_p) and _p not in sys.path:
        sys.path.append(_p)

import concourse.bacc as bacc
import concourse.bass as bass
import concourse.mybir as mybir
import concourse.tile as tile
from concourse.bass_utils import run_bass_kernel_spmd

F32 = mybir.dt.float32
F32R = mybir.dt.float32r

N = 4096          # IN_F == OUT_F == N_PERM == DIAG
B = 256           # batch
NCORES = 8
RW = N // NCORES  # 512 output columns per core
K_TOPK = 3687     # ceil(0.9 * 4096 * 4096 / 4096)
CB = 128          # contraction block (SBUF partition count)
NCB = N // CB     # 32 contraction blocks
TB = 4            # contraction blocks per DMA/multiply batch
GW = N + RW - CB  # 4480: width of the Toeplitz scale master tile


def _strided_cols(ap2d, col_off, t_step, n_t, inner):
    """[128, W] SBUF tile -> [128, n_t, inner] view starting at col_off with
    column stride t_step between t-slices (overlap allowed)."""
    pstep = ap2d.ap[0][0]
    return bass.AP(
        ap2d.tensor, ap2d.offset + col_off,
        [[pstep, 128], [t_step, n_t], [1, inner]],
    )


def _build_program():
    nc = bacc.Bacc("TRN2", target_bir_lowering=False, debug=False)

    band = nc.dram_tensor("band", [N, RW], F32, kind="ExternalInput").ap()
    xT = nc.dram_tensor("xT", [N, B], F32R, kind="ExternalInput").ap()
    alpha_r = nc.dram_tensor("alpha_r", [N], F32, kind="ExternalInput").ap()
    out = nc.dram_tensor("out", [B, RW], F32, kind="ExternalOutput").ap()

    with tile.TileContext(nc) as tc:
        with (
            tc.tile_pool(name="small", bufs=1) as sp,
            tc.tile_pool(name="gpool", bufs=1) as gp,
            tc.tile_pool(name="dram", bufs=1, space="DRAM") as dp,
            tc.tile_pool(name="vb", bufs=3) as vbp,
            tc.tile_pool(name="wt", bufs=3) as wtp,
            tc.tile_pool(name="xt", bufs=3) as xtp,
            tc.tile_pool(name="opool", bufs=2) as op,
            tc.tile_pool(name="psum", bufs=1, space="PSUM") as pp,
            tc.tile_pool(name="psum_s", bufs=1, space="PSUM") as pps,
        ):
            # ---- Phase 0: a = min(K * softmax(alpha), 1); build G ----
            alpha_sb = sp.tile([128, N // 128], F32)
            nc.sync.dma_start(alpha_sb[:], alpha_r.rearrange("(p f) -> p f", p=128))

            exp_sb = sp.tile([128, N // 128], F32)
            rowsum = sp.tile([128, 1], F32)
            # alpha is uniform in [0,1): no max-subtraction needed for stability
            nc.scalar.activation(
                exp_sb[:], alpha_sb[:], mybir.ActivationFunctionType.Exp,
                accum_out=rowsum[:],
            )

            ones = sp.tile([128, 128], F32)
            nc.vector.memset(ones[:], 1.0)
            tot_ps = pps.tile([128, 1], F32)
            # total = ones.T @ rowsum -> per-partition copy of the full sum
            nc.tensor.matmul(tot_ps[:], ones[:], rowsum[:], start=True, stop=True)
            inv_sb = sp.tile([128, 1], F32)
            nc.vector.reciprocal(inv_sb[:], tot_ps[:])

            a_sb = sp.tile([128, N // 128], F32)
            nc.vector.tensor_scalar(
                a_sb[:], exp_sb[:], inv_sb[:], float(K_TOPK),
                mybir.AluOpType.mult, mybir.AluOpType.mult,
            )
            nc.vector.tensor_scalar_min(a_sb[:], a_sb[:], 1.0)

            # doubled copy of a in DRAM so the Toeplitz windows never wrap
            a2 = dp.tile([2 * N], F32)
            nc.sync.dma_start(a2[0:N].rearrange("(p f) -> p f", p=128), a_sb[:])
            nc.sync.dma_start(a2[N:2 * N].rearrange("(p f) -> p f", p=128), a_sb[:])

            # G[c_l, t] = a2[1 + c_l + t]: overlapping windows, +1 partition step
            g_tile = gp.tile([128, GW], F32)
            g_src = bass.AP(a2.tensor, a2.offset + 1, [[1, 128], [1, GW]])
            nc.gpsimd.dma_start(g_tile[:], g_src)

            # ---- Phase 1: NCB/TB batches of (band * G-slices) matmuls ----
            psum0 = pp.tile([128, RW], F32)
            psum1 = pp.tile([128, RW], F32)
            for q0 in range(0, NCB, TB):
                rows = slice(q0 * CB, (q0 + TB) * CB)
                vb = vbp.tile([128, TB, RW], F32)
                nc.sync.dma_start(
                    vb[:], band[rows, :].rearrange("(t p) j -> p t j", p=128)
                )
                xt = xtp.tile([128, TB, B], F32R)
                nc.sync.dma_start(
                    xt[:], xT[rows, :].rearrange("(t p) j -> p t j", p=128)
                )

                wt = wtp.tile([128, TB, RW], F32R)
                for t in range(TB):
                    off = (q0 + t) * CB
                    nc.vector.tensor_tensor(
                        wt[:, t, :], vb[:, t, :], g_tile[:, off:off + RW],
                        mybir.AluOpType.mult,
                    )
                for t in range(TB):
                    k = q0 + t
                    nc.tensor.matmul(psum0[:], xt[:, t, 0:128], wt[:, t, :],
                                     start=(k == 0), stop=(k == NCB - 1))
                    nc.tensor.matmul(psum1[:], xt[:, t, 128:256], wt[:, t, :],
                                     start=(k == 0), stop=(k == NCB - 1))

            # ---- Phase 2: PSUM -> SBUF -> DRAM ----
            o0 = op.tile([128, RW], F32)
            nc.vector.tensor_copy(o0[:], psum0[:])
            nc.sync.dma_start(out[0:128, :], o0[:])
            o1 = op.tile([128, RW], F32)
            nc.vector.tensor_copy(o1[:], psum1[:])
            nc.sync.dma_start(out[128:256, :], o1[:])

    nc.compile()
    return nc


_NC_CACHE = []


def _get_program():
    if not _NC_CACHE:
        _NC_CACHE.append(_build_program())
    return _NC_CACHE[0]


def prepare_in_maps(x: np.ndarray, V: np.ndarray, alpha: np.ndarray):
    """Layout-only sharding of the full inputs into 8 per-core input maps."""
    x = np.ascontiguousarray(np.asarray(x, dtype=np.float32))
    V = np.ascontiguousarray(np.asarray(V, dtype=np.float32))
    alpha = np.ascontiguousarray(np.asarray(alpha, dtype=np.float32))

    # rows presented in reversed order (c = N-1-p); see G-tile comment
    xT = np.ascontiguousarray(x.T[::-1, :])  # (N, B)

    # VtD[c, t] = V[t % N, c] for t in [0, 2N): doubled transpose for wrap-free
    # band extraction. band_m[c, j] = V[(r0 + j - c) % N, c]
    #              = VtD[c, N + r0 + j - c]
    Vt = np.ascontiguousarray(V.T)
    VtD = np.concatenate([Vt, Vt], axis=1)  # (N, 2N)
    flat = VtD.reshape(-1)
    isz = flat.itemsize

    in_maps = []
    for m in range(NCORES):
        r0 = m * RW
        start = N + r0  # element offset of band_m[0, 0] in flat
        band_m = np.lib.stride_tricks.as_strided(
            flat[start:], shape=(N, RW), strides=((2 * N - 1) * isz, isz),
        )
        in_maps.append({
            "band": np.ascontiguousarray(band_m[::-1, :]),
            "xT": xT,
            "alpha_r": np.ascontiguousarray(np.roll(alpha, -r0)),
        })
    return in_maps


def gather_output(results) -> np.ndarray:
    return np.concatenate([results[m]["out"] for m in range(NCORES)], axis=1)


def kernel(x: np.ndarray, V: np.ndarray, alpha: np.ndarray) -> np.ndarray:
    in_maps = prepare_in_maps(x, V, alpha)
    nc = _get_program()
    res = run_bass_kernel_spmd(nc, in_maps, core_ids=list(range(NCORES)))
    return gather_output(res.results)
